# revision 1
# baseline (speedup 1.0000x reference)
"""Trainium2 Bass kernel for nn_ByteSequenceEmbedder.

Data-parallel across 8 NeuronCores: 2 sequences per core, weights replicated.

Per-core dataflow (all activations channels-on-partitions, "layout A" [C, T]):
  embed   : one-hot matmul — tokens broadcast [128,T] (host), DVE is_equal vs
            per-partition iota -> onehot chunks; PE: tok_emb-chunks.T @ onehot
            accumulated in PSUM (+ K=1 matmul adding bpe-marker row)
  conv0   : 3 shifted matmuls per (T-chunk, co-chunk) accumulating in PSUM,
            ReLU+bias fused into the ACT PSUM->SBUF evacuation
  highway : 2 blocks x 2 layers; 8x4 matmuls per T-chunk, ReLU/Sigmoid evac,
            DVE combine x' = g*(relu(h)-x)+x
  conv1   : 12 matmuls per (T-chunk, co-chunk) + residual add
  pool    : ragged word max-pool as masked shifted max:
            msel[t] = max(x2[t], x2[t+1]+A1[t], x2[t+2]+A2[t]) with host-built
            additive masks (0 where word@t has len>j, else -1e30)
  proj    : projection applied over ALL T positions; host selects column s_w
            per word while unsharding (empty pools -> proj_b row)

Matmul operands are bf16 (f32 PSUM accumulation).
"""
import numpy as np

import concourse.bacc as bacc
import concourse.tile as tile
import concourse.mybir as mybir

BSZ, NW, T = 16, 1024, 3072
BED, WED = 128, 512
VOCAB = 264
BPE_MASK_IDX = 4
N_CORES = 8
SEQ_PER_CORE = BSZ // N_CORES
TP = T + 2          # conv buffers: one zero halo col each side
TP2 = T + 4         # pooling source: 1 left + 3 right halo cols
NCH = T // 512      # T-chunks of 512
BF16 = mybir.dt.bfloat16
F16 = mybir.dt.float16
F32 = mybir.dt.float32

_BF16_NP = mybir.dt.np(BF16)
_F16_NP = np.float16
NEG_BIG = -1e30

_CACHE = {}


def _build_program():
    nc = bacc.Bacc("TRN2", target_bir_lowering=False, debug=False)

    def dram_in(name, shape, dt):
        return nc.dram_tensor(name, shape, dt, kind="ExternalInput").ap()

    emb_lhs = dram_in("emb_lhs", [128, 3 * 128], BF16)   # tok_emb row-chunks
    emb_row4 = dram_in("emb_row4", [1, 128], BF16)       # tok_emb[4]
    iota_c = dram_in("iota_c", [128, 3], F32)            # per-partition vocab iota
    w_c0 = dram_in("w_c0", [128, 3 * WED], BF16)         # [ci, k*512+co]
    w_c1 = dram_in("w_c1", [128, 4 * 3 * WED], BF16)     # [ci%128, (q*3+k)*512+co]
    w_hw = dram_in("w_hw", [128, 4 * 4 * 1024], BF16)    # [(bl*4+q)*1024 + co_out]
    w_pr = dram_in("w_pr", [128, 4 * WED], BF16)         # [q*512+co]
    b_c0 = dram_in("b_c0", [128, 4], F32)
    b_c1 = dram_in("b_c1", [128, 4], F32)
    b_hw = dram_in("b_hw", [128, 4 * 8], F32)            # [bl*8 + m]
    b_pr = dram_in("b_pr", [128, 4], F32)
    tok_bc = dram_in("tok_bc", [SEQ_PER_CORE, 128, T], F16)   # tokens bcast over partitions
    bpe_row = dram_in("bpe_row", [SEQ_PER_CORE, 1, T], BF16)  # bpe mask 0/1
    a_msk = dram_in("a_msk", [SEQ_PER_CORE, 128, 2 * T], BF16)  # pooling additive masks

    out = nc.dram_tensor("out", [SEQ_PER_CORE, WED, T], F32, kind="ExternalOutput").ap()

    RELU = mybir.ActivationFunctionType.Relu
    SIGM = mybir.ActivationFunctionType.Sigmoid
    IDEN = mybir.ActivationFunctionType.Identity
    MAX = mybir.AluOpType.max
    ADD = mybir.AluOpType.add
    SUB = mybir.AluOpType.subtract
    MUL = mybir.AluOpType.mult
    ISEQ = mybir.AluOpType.is_equal

    with tile.TileContext(nc) as tc:
        with tc.tile_pool(name="wp", bufs=1) as wp, \
             tc.tile_pool(name="ap", bufs=1) as apool, \
             tc.tile_pool(name="tp", bufs=3) as tp, \
             tc.tile_pool(name="pp", bufs=8, space="PSUM") as pp:

            # ---- HAM warm-up: PE activity from t~0 so real matmuls start at 2.4GHz ----
            wu = wp.tile([128, 512], BF16)
            nc.vector.memset(wu[:], 0)
            for _ in range(20):
                wps = pp.tile([128, 512], F32, tag="ps", name="wps")
                nc.tensor.matmul(out=wps[:], lhsT=wu[:, 0:128], rhs=wu[:],
                                 start=True, stop=True)

            # ---- load weights/biases once (sync queue; small/early first) ----
            t_embA = wp.tile([128, 3 * 128], BF16)
            t_row4 = wp.tile([1, 128], BF16)
            t_iota = wp.tile([128, 3], F32)
            t_bc0 = wp.tile([128, 4], F32)
            t_bc1 = wp.tile([128, 4], F32)
            t_bhw = wp.tile([128, 4 * 8], F32)
            t_bpr = wp.tile([128, 4], F32)
            t_wc0 = wp.tile([128, 3 * WED], BF16)
            t_wc1 = wp.tile([128, 4 * 3 * WED], BF16)
            t_whw = wp.tile([128, 4 * 4 * 1024], BF16)
            t_wpr = wp.tile([128, 4 * WED], BF16)
            # first token chunk for seq 0 ahead of the weight loads (critical path
            # to the very first real matmul); rest of seq-0 embed inputs follow on
            # the scalar queue inside embed_seq.
            t_tok0 = apool.tile([128, T], F16, tag="tok", name="t_tok0", bufs=2)
            nc.sync.dma_start(out=t_tok0[:, 0:512], in_=tok_bc[0, :, 0:512])
            for t, d in ((t_embA, emb_lhs), (t_row4, emb_row4), (t_iota, iota_c),
                         (t_bc0, b_c0), (t_bc1, b_c1), (t_bhw, b_hw), (t_bpr, b_pr),
                         (t_wc0, w_c0), (t_wc1, w_c1), (t_whw, w_hw), (t_wpr, w_pr)):
                nc.sync.dma_start(out=t[:], in_=d[:])

            def conv_block(X, Y, wt, bt, n_ci):
                """Y[:, chunk m cols 1..T] = relu(conv(X) + b)."""
                for n in range(NCH):
                    for m in range(4):
                        ps = pp.tile([128, 512], F32, tag="ps", name="ps")
                        nmm = n_ci * 3
                        i = 0
                        for q in range(n_ci):
                            for k in range(3):
                                lhs = wt[:, (q * 3 + k) * WED + m * 128:(q * 3 + k) * WED + (m + 1) * 128]
                                nc.tensor.matmul(
                                    out=ps[:], lhsT=lhs,
                                    rhs=X[:, q * TP + n * 512 + k:q * TP + n * 512 + k + 512],
                                    start=(i == 0), stop=(i == nmm - 1))
                                i += 1
                        dst = Y[:, m * TP + 1 + n * 512:m * TP + 1 + (n + 1) * 512]
                        nc.scalar.activation(out=dst, in_=ps[:], func=RELU,
                                             bias=bt[:, m:m + 1], scale=1.0)

            def highway_layer(X, Y, bl, ytp=TP):
                """Y = g*relu(h) + (1-g)*X; X [128, 4*TP], Y [128, 4*ytp]."""
                for n in range(NCH):
                    pss = []
                    for m in range(8):
                        ps = pp.tile([128, 512], F32, tag="ps", name="ps")
                        for q in range(4):
                            base = (bl * 4 + q) * 1024 + m * 128
                            nc.tensor.matmul(
                                out=ps[:], lhsT=t_whw[:, base:base + 128],
                                rhs=X[:, q * TP + 1 + n * 512:q * TP + 1 + (n + 1) * 512],
                                start=(q == 0), stop=(q == 3))
                        pss.append(ps)
                    for c in range(4):
                        xs = X[:, c * TP + 1 + n * 512:c * TP + 1 + (n + 1) * 512]
                        h_t = tp.tile([128, 512], BF16, tag="h", name="h_t")
                        g_t = tp.tile([128, 512], BF16, tag="g", name="g_t")
                        d_t = tp.tile([128, 512], BF16, tag="d", name="d_t")
                        nc.scalar.activation(out=h_t[:], in_=pss[c][:], func=RELU,
                                             bias=t_bhw[:, bl * 8 + c:bl * 8 + c + 1], scale=1.0)
                        nc.scalar.activation(out=g_t[:], in_=pss[4 + c][:], func=SIGM,
                                             bias=t_bhw[:, bl * 8 + 4 + c:bl * 8 + 4 + c + 1], scale=1.0)
                        nc.vector.tensor_tensor(out=d_t[:], in0=h_t[:], in1=xs, op=SUB)
                        nc.vector.tensor_tensor(out=d_t[:], in0=d_t[:], in1=g_t[:], op=MUL)
                        ys = Y[:, c * ytp + 1 + n * 512:c * ytp + 1 + (n + 1) * 512]
                        nc.vector.tensor_tensor(out=ys, in0=d_t[:], in1=xs, op=ADD)

            scope = nc.named_scope

            def embed_seq(s, t_tok=None):
                """One-hot-matmul embedding for sequence s -> x0 [128, TP] bf16.
                Token DMA is chunked so the first compare starts early."""
                ctx = scope(f"s{s}_embed"); ctx.__enter__()
                skip0 = t_tok is not None
                if t_tok is None:
                    t_tok = apool.tile([128, T], F16, tag="tok", name="t_tok", bufs=2)
                t_bpe = apool.tile([1, T], BF16, tag="bpe", name="t_bpe", bufs=2)
                t_am = apool.tile([128, 2 * T], BF16, tag="am", name="t_am")
                for n in range(1 if skip0 else 0, NCH):
                    nc.scalar.dma_start(out=t_tok[:, n * 512:(n + 1) * 512],
                                        in_=tok_bc[s, :, n * 512:(n + 1) * 512])
                nc.scalar.dma_start(out=t_bpe[:], in_=bpe_row[s])
                nc.scalar.dma_start(out=t_am[:], in_=a_msk[s])

                x0 = apool.tile([128, TP], BF16, tag="x0", name="x0", bufs=2)
                nc.vector.memset(x0[:, 0:1], 0)
                nc.vector.memset(x0[:, TP - 1:TP], 0)
                for n in range(NCH):
                    oh1 = tp.tile([128, 512], BF16, tag="oh1", name="oh1")
                    oh2 = tp.tile([128, 512], BF16, tag="oh2", name="oh2")
                    oh3 = tp.tile([8, 512], BF16, tag="oh3", name="oh3")
                    tb = t_tok[:, n * 512:(n + 1) * 512]
                    nc.vector.tensor_scalar(out=oh1[:], in0=tb, scalar1=t_iota[:, 0:1],
                                            scalar2=None, op0=ISEQ)
                    nc.vector.tensor_scalar(out=oh2[:], in0=tb, scalar1=t_iota[:, 1:2],
                                            scalar2=None, op0=ISEQ)
                    nc.vector.tensor_scalar(out=oh3[:], in0=t_tok[0:8, n * 512:(n + 1) * 512],
                                            scalar1=t_iota[0:8, 2:3], scalar2=None, op0=ISEQ)
                    ps = pp.tile([128, 512], F32, tag="ps", name="ps")
                    nc.tensor.matmul(out=ps[:], lhsT=t_embA[:, 0:128], rhs=oh1[:],
                                     start=True, stop=False)
                    nc.tensor.matmul(out=ps[:], lhsT=t_embA[:, 128:256], rhs=oh2[:],
                                     start=False, stop=False)
                    nc.tensor.matmul(out=ps[:], lhsT=t_embA[0:8, 256:384], rhs=oh3[:],
                                     start=False, stop=False)
                    nc.tensor.matmul(out=ps[:], lhsT=t_row4[:], rhs=t_bpe[:, n * 512:(n + 1) * 512],
                                     start=False, stop=True)
                    nc.scalar.activation(out=x0[:, 1 + n * 512:1 + (n + 1) * 512],
                                         in_=ps[:], func=IDEN, bias=0.0, scale=1.0)
                ctx.__exit__(None, None, None)
                return x0, t_am

            x0, t_am = embed_seq(0, t_tok=t_tok0)
            x1 = None
            for s in range(SEQ_PER_CORE):

                def act_buf(tag, w=TP, extra_halo=0):
                    b = apool.tile([128, 4 * w], BF16, tag=tag, name=tag)
                    for q in range(4):
                        nc.vector.memset(b[:, q * w:q * w + 1], 0)
                        nc.vector.memset(b[:, q * w + 1 + T:(q + 1) * w], 0)
                    return b

                # ---------- conv0 + highway block 0 ----------
                if x1 is None:
                    with scope(f"s{s}_conv0"):
                        x1 = act_buf("actA")
                        conv_block(x0, x1, t_wc0, t_bc0, 1)
                with scope(f"s{s}_hw0l0"):
                    x1b = act_buf("actB")
                    highway_layer(x1, x1b, 0)
                with scope(f"s{s}_hw0l1"):
                    x1c = act_buf("actC")
                    highway_layer(x1b, x1c, 1)

                # ---------- conv1 (+res) + highway block 1 ----------
                ctx = scope(f"s{s}_conv1"); ctx.__enter__()
                x2p = act_buf("actA")
                for n in range(NCH):
                    for m in range(4):
                        ps = pp.tile([128, 512], F32, tag="ps", name="ps")
                        i = 0
                        for q in range(4):
                            for k in range(3):
                                lhs = t_wc1[:, (q * 3 + k) * WED + m * 128:(q * 3 + k) * WED + (m + 1) * 128]
                                nc.tensor.matmul(
                                    out=ps[:], lhsT=lhs,
                                    rhs=x1c[:, q * TP + n * 512 + k:q * TP + n * 512 + k + 512],
                                    start=(i == 0), stop=(i == 11))
                                i += 1
                        r_t = tp.tile([128, 512], BF16, tag="h", name="r_t")
                        nc.scalar.activation(out=r_t[:], in_=ps[:], func=RELU,
                                             bias=t_bc1[:, m:m + 1], scale=1.0)
                        xs = x1c[:, m * TP + 1 + n * 512:m * TP + 1 + (n + 1) * 512]
                        nc.vector.tensor_tensor(
                            out=x2p[:, m * TP + 1 + n * 512:m * TP + 1 + (n + 1) * 512],
                            in0=r_t[:], in1=xs, op=ADD)
                ctx.__exit__(None, None, None)

                with scope(f"s{s}_hw1l0"):
                    x2b = act_buf("actB")
                    highway_layer(x2p, x2b, 2)
                with scope(f"s{s}_hw1l1"):
                    x2 = act_buf("actC", w=TP2)
                    highway_layer(x2b, x2, 3, ytp=TP2)

                # prefetch next sequence's embedding + conv0: fills the PE bubbles of
                # this sequence's DVE-bound pool phase
                if s + 1 < SEQ_PER_CORE:
                    next_x0, next_am = embed_seq(s + 1)
                    with scope(f"s{s + 1}_conv0"):
                        next_x1 = act_buf("actA")
                        conv_block(next_x0, next_x1, t_wc0, t_bc0, 1)

                # ---------- ragged max pool + projection, pipelined per T-chunk ----------
                ctx = scope(f"s{s}_poolproj"); ctx.__enter__()
                msel = apool.tile([128, 4 * T], BF16, tag="actB", name="msel")
                for n in range(NCH):
                    lo, hi = n * 512, (n + 1) * 512
                    for c in range(4):
                        base = c * TP2 + 1
                        s1 = tp.tile([128, 512], BF16, tag="s1", name="s1")
                        s2 = tp.tile([128, 512], BF16, tag="s2", name="s2")
                        nc.vector.tensor_tensor(out=s1[:], in0=x2[:, base + 1 + lo:base + 1 + hi],
                                                in1=t_am[:, lo:hi], op=ADD)
                        nc.vector.tensor_tensor(out=s2[:], in0=x2[:, base + 2 + lo:base + 2 + hi],
                                                in1=t_am[:, T + lo:T + hi], op=ADD)
                        nc.vector.tensor_tensor(out=s1[:], in0=s1[:], in1=s2[:], op=MAX)
                        nc.vector.tensor_tensor(out=msel[:, c * T + lo:c * T + hi],
                                                in0=s1[:], in1=x2[:, base + lo:base + hi], op=MAX)
                    for m in range(4):
                        ps = pp.tile([128, 512], F32, tag="ps", name="ps")
                        for q in range(4):
                            nc.tensor.matmul(
                                out=ps[:], lhsT=t_wpr[:, q * WED + m * 128:q * WED + (m + 1) * 128],
                                rhs=msel[:, q * T + lo:q * T + hi],
                                start=(q == 0), stop=(q == 3))
                        o_t = tp.tile([128, 512], F32, tag="o", name="o_t", bufs=4)
                        nc.scalar.activation(out=o_t[:], in_=ps[:], func=IDEN,
                                             bias=t_bpr[:, m:m + 1], scale=1.0)
                        nc.sync.dma_start(out=out[s, m * 128:(m + 1) * 128, lo:hi], in_=o_t[:])
                ctx.__exit__(None, None, None)
                if s + 1 < SEQ_PER_CORE:
                    x0, t_am, x1 = next_x0, next_am, next_x1
                else:
                    x1 = None

    nc.compile()
    return nc


def _prep_inputs(inputs):
    """Host-side: shard + convert to the kernel's DRAM tensor layouts."""
    byte_tokens = np.asarray(inputs["byte_tokens"], np.int64)
    bpe_mask = np.asarray(inputs["bpe_mask"], bool)
    pool_lengths = np.asarray(inputs["pool_lengths"], np.int64)
    tok_emb = np.asarray(inputs["tok_emb"], np.float32)

    def bf(x):
        return np.ascontiguousarray(np.asarray(x, np.float32).astype(_BF16_NP))

    conv0_W = np.asarray(inputs["conv0_W"], np.float32)   # [3,128,512]
    conv1_W = np.asarray(inputs["conv1_W"], np.float32)   # [3,512,512]
    hw0_W = np.asarray(inputs["hw0_W"], np.float32)       # [2,1024,512]
    hw1_W = np.asarray(inputs["hw1_W"], np.float32)
    proj_W = np.asarray(inputs["proj_W"], np.float32)     # [512,512]

    w_c0 = bf(conv0_W.transpose(1, 0, 2).reshape(128, 3 * WED))
    w_c1 = bf(conv1_W.transpose(1, 0, 2).reshape(4, 128, 3, WED)
              .transpose(1, 0, 2, 3).reshape(128, 4 * 3 * WED))
    whw = np.empty((128, 16, 1024), np.float32)
    for bl, (blk, lay) in enumerate(((hw0_W, 0), (hw0_W, 1), (hw1_W, 0), (hw1_W, 1))):
        wt = blk[lay].T  # [512, 1024]
        for q in range(4):
            whw[:, bl * 4 + q, :] = wt[q * 128:(q + 1) * 128]
    w_hw = bf(whw.reshape(128, 16 * 1024))
    w_pr = bf(proj_W.T.reshape(4, 128, WED).transpose(1, 0, 2).reshape(128, 4 * WED))

    def colchunks(b):  # [512] -> [128, 4]
        return np.ascontiguousarray(np.asarray(b, np.float32).reshape(4, 128).T)

    b_c0 = colchunks(inputs["conv0_b"])
    b_c1 = colchunks(inputs["conv1_b"])
    bhw = np.empty((128, 4, 8), np.float32)
    for bl, (blk, lay) in enumerate((("hw0_b", 0), ("hw0_b", 1), ("hw1_b", 0), ("hw1_b", 1))):
        b = np.asarray(inputs[blk], np.float32)[lay]      # [1024]
        bhw[:, bl, 0:4] = b[:512].reshape(4, 128).T
        bhw[:, bl, 4:8] = b[512:1024].reshape(4, 128).T
    b_hw = np.ascontiguousarray(bhw.reshape(128, 32))
    b_pr = colchunks(inputs["proj_b"])

    # embedding table as lhsT row-chunks [128, 3*128]
    emb_lhs = np.zeros((128, 3 * 128), np.float32)
    emb_lhs[:, 0:128] = tok_emb[0:128]
    emb_lhs[:, 128:256] = tok_emb[128:256]
    emb_lhs[0:8, 256:384] = tok_emb[256:264]
    emb_lhs = bf(emb_lhs)
    emb_row4 = bf(tok_emb[BPE_MASK_IDX:BPE_MASK_IDX + 1, :])  # [1, 128]
    iota_c = np.empty((128, 3), np.float32)
    p = np.arange(128)
    for j in range(3):
        iota_c[:, j] = (j * 128 + p).astype(np.float32)

    shared = dict(emb_lhs=emb_lhs, emb_row4=emb_row4, iota_c=iota_c,
                  w_c0=w_c0, w_c1=w_c1, w_hw=w_hw, w_pr=w_pr,
                  b_c0=b_c0, b_c1=b_c1, b_hw=b_hw, b_pr=b_pr)

    in_maps = []
    meta = []
    for core in range(N_CORES):
        m = dict(shared)
        tok = np.empty((SEQ_PER_CORE, 128, T), _F16_NP)
        bpe = np.empty((SEQ_PER_CORE, 1, T), _BF16_NP)
        amsk = np.empty((SEQ_PER_CORE, 128, 2 * T), _BF16_NP)
        for s in range(SEQ_PER_CORE):
            b = core * SEQ_PER_CORE + s
            tok[s] = np.broadcast_to(byte_tokens[b].astype(_F16_NP), (128, T))
            bpe[s, 0] = (bpe_mask[b]).astype(_BF16_NP)
            pl = pool_lengths[b]
            cum = np.cumsum(pl)
            s_w = (cum - pl)
            a1 = np.full(T, NEG_BIG, np.float32)
            a2 = np.full(T, NEG_BIG, np.float32)
            st = s_w[pl > 1]
            a1[st[st < T]] = 0.0
            st = s_w[pl > 2]
            a2[st[st < T]] = 0.0
            amsk[s, :, 0:T] = np.broadcast_to(a1.astype(_BF16_NP), (128, T))
            amsk[s, :, T:2 * T] = np.broadcast_to(a2.astype(_BF16_NP), (128, T))
            meta.append((s_w, pl))
        m["tok_bc"] = tok
        m["bpe_row"] = bpe
        m["a_msk"] = amsk
        in_maps.append(m)
    return in_maps, meta


def kernel(**inputs) -> np.ndarray:
    from concourse.bass_utils import run_bass_kernel_spmd

    if "nc" not in _CACHE:
        _CACHE["nc"] = _build_program()
    nc = _CACHE["nc"]

    in_maps, meta = _prep_inputs(inputs)
    res = run_bass_kernel_spmd(nc, in_maps, list(range(N_CORES)))

    proj_b = np.asarray(inputs["proj_b"], np.float32)
    full = np.empty((BSZ, NW, WED), np.float32)
    for core in range(N_CORES):
        o = np.asarray(res.results[core]["out"], np.float32)  # [2, 512, T]
        for s in range(SEQ_PER_CORE):
            b = core * SEQ_PER_CORE + s
            s_w, pl = meta[b]
            cols = np.clip(s_w, 0, T - 1)
            full[b] = o[s][:, cols].T
            if (pl == 0).any():
                full[b][pl == 0] = proj_b
    return full



# revision 16
# speedup vs baseline: 1.1540x; 1.1540x over previous
"""Trainium2 Bass kernel for nn_ByteSequenceEmbedder.

Data-parallel across 8 NeuronCores: 2 sequences per core, weights replicated.

v2: the two sequences are packed into ONE column strip of W = 2*R columns
(R = 2112 >= max src_len 2085 + halos), cutting all per-column PE work by
~31% vs the padded T=3072 layout.  The embedding lookup is precomputed on
the host (same DMA bytes as shipping broadcast tokens) and both k=3 convs
use Winograd F(2,3): four shared m-term matmul groups per output pair
(m0=G0@u0, m1=Ga@u1, m2=Gb@u2, m3=G3@u3; y_even=m0+m1+m2,
y_odd=m1-m2-m3), 2/3 of the direct-conv MACs.

Strip layout per core (strip cols 0..W-1, region s in {0,1} at [s*R,(s+1)*R)):
  col s*R            "Z" col: zero in x0 (host) and FORCED zero in x1c so
                     conv1 of region s sees the SAME-pad zero at t=-1
  cols s*R+1 ..      seq content (src_s cols); the col after the content is
                     computed naturally from x0=0 neighborhoods and matches
                     the reference values at t=src_s (conv1 right halo)
Pooling uses host-built additive masks over strip cols; host selects the
word-start columns from the [512, W] output.
"""
import numpy as np

import concourse.bacc as bacc
import concourse.tile as tile
import concourse.mybir as mybir

BSZ, NW, T = 16, 1024, 3072
BED, WED = 128, 512
BPE_MASK_IDX = 4
N_CORES = 8
SEQ_PER_CORE = BSZ // N_CORES
BF16 = mybir.dt.bfloat16
F32 = mybir.dt.float32

_BF16_NP = mybir.dt.np(BF16)
NEG_BIG = -1e30

_CACHE = {}


def _chunks(total, step):
    out = []
    lo = 0
    while lo < total:
        out.append((lo, min(step, total - lo)))
        lo += min(step, total - lo)
    return out


def _build_program(R):
    W = 2 * R
    Wp = W + 4        # x0 buffer: 1 left halo + 3 right halo cols
    Wq = W + 4        # act buffers: 1 left halo + 3 right halo per co-chunk
    J = W // 2        # winograd output pairs
    CH = _chunks(W, 512)    # output chunks
    JCH = _chunks(J, 256)   # pair chunks (256 keeps the u-tiles small)

    nc = bacc.Bacc("TRN2", target_bir_lowering=False, debug=False)

    def dram_in(name, shape, dt):
        return nc.dram_tensor(name, shape, dt, kind="ExternalInput").ap()

    # winograd conv weights: [128ci, ((v*nci+q)*4+m)*128+j] , v in {G0,Ga,Gb,G3}
    w_c0 = dram_in("w_c0", [128, 4 * 1 * WED], BF16)
    w_c1 = dram_in("w_c1", [128, 4 * 4 * WED], BF16)
    w_hw = dram_in("w_hw", [128, 4 * 4 * 1024], BF16)   # [(bl*4+q)*1024 + co]
    w_pr = dram_in("w_pr", [128, 4 * WED], BF16)
    b_c0 = dram_in("b_c0", [128, 4], F32)
    b_c1 = dram_in("b_c1", [128, 4], F32)
    b_hw = dram_in("b_hw", [128, 4 * 8], F32)           # [bl*8 + m]
    b_pr = dram_in("b_pr", [128, 4], F32)
    x0_in = dram_in("x0_in", [128, W], BF16)            # host-precomputed embedding
    a_msk = dram_in("a_msk", [128, 2 * W], BF16)        # pooling additive masks

    out = nc.dram_tensor("out", [WED, W], BF16, kind="ExternalOutput").ap()

    RELU = mybir.ActivationFunctionType.Relu
    SIGM = mybir.ActivationFunctionType.Sigmoid
    IDEN = mybir.ActivationFunctionType.Identity
    MAX = mybir.AluOpType.max
    ADD = mybir.AluOpType.add
    SUB = mybir.AluOpType.subtract
    MUL = mybir.AluOpType.mult

    with tile.TileContext(nc) as tc:
        with tc.tile_pool(name="wp", bufs=1) as wp, \
             tc.tile_pool(name="ap", bufs=1) as apool, \
             tc.tile_pool(name="tp", bufs=3) as tp, \
             tc.tile_pool(name="up", bufs=2) as upool, \
             tc.tile_pool(name="pp", bufs=8, space="PSUM") as pp:

            # ---- HAM warm-up: PE activity from t~0 so real matmuls start fast ----
            wu = wp.tile([128, 512], BF16)
            nc.vector.memset(wu[:], 0)
            for _ in range(20):
                wps = pp.tile([128, 512], F32, tag="ps", name="wps")
                nc.tensor.matmul(out=wps[:], lhsT=wu[:, 0:128], rhs=wu[:],
                                 start=True, stop=True)

            # ---- load weights/biases/inputs (multiple queues; early needs first) ----
            t_bc0 = wp.tile([128, 4], F32)
            t_bc1 = wp.tile([128, 4], F32)
            t_bhw = wp.tile([128, 4 * 8], F32)
            t_bpr = wp.tile([128, 4], F32)
            t_wc0 = wp.tile([128, 4 * WED], BF16)
            t_wc1 = wp.tile([128, 4 * 4 * WED], BF16)
            t_whw = wp.tile([128, 4 * 4 * 1024], BF16)
            t_wpr = wp.tile([128, 4 * WED], BF16)

            # x0 strip with halo cols; content at buffer cols 1..W
            t_x0 = apool.tile([128, Wp], BF16, tag="actC", name="t_x0")
            nc.vector.memset(t_x0[:, 0:1], 0)
            nc.vector.memset(t_x0[:, W + 1:W + 4], 0)
            # x0 pieces on the scalar queue so conv0 can start immediately
            for lo, cw in CH:
                nc.scalar.dma_start(out=t_x0[:, 1 + lo:1 + lo + cw],
                                    in_=x0_in[:, lo:lo + cw])
            # sync queue: conv0 weights + biases, then the big highway weights
            for t, d in ((t_bc0, b_c0), (t_wc0, w_c0), (t_bc1, b_c1),
                         (t_bhw, b_hw), (t_bpr, b_pr), (t_whw, w_hw)):
                nc.sync.dma_start(out=t[:], in_=d[:])
            # gpsimd queue: conv1 + proj weights
            nc.gpsimd.dma_start(out=t_wc1[:], in_=w_c1[:])
            nc.gpsimd.dma_start(out=t_wpr[:], in_=w_pr[:])

            def act_buf(tag):
                b = apool.tile([128, 4 * Wq], BF16, tag=tag, name=tag)
                for q in range(4):
                    nc.vector.memset(b[:, q * Wq:q * Wq + 1], 0)
                    nc.vector.memset(b[:, q * Wq + 1 + W:(q + 1) * Wq], 0)
                return b

            def wino_conv(X, xbase, Y, wt, bt, n_ci, residual=None):
                """Y[:, q*Wq+1 .. +W] (+res) = relu(conv3(X)+b) via F(2,3).

                X: input buffer, xbase(q) -> column of strip col 0 for ci-chunk q.
                Y: output act buffer (4 co-chunks). residual: act buffer or None.
                """
                for j0, jw in JCH:
                    # input transforms: d_k = strip col 2j-1+k = X col xbase+2j+k
                    ut = upool.tile([128, 4 * n_ci * 256], BF16, tag="u",
                                    name="ut")
                    for q in range(n_ci):
                        b = xbase(q) + 2 * j0
                        d0 = X[:, b:b + 2 * jw:2]
                        d1 = X[:, b + 1:b + 1 + 2 * jw:2]
                        d2 = X[:, b + 2:b + 2 + 2 * jw:2]
                        d3 = X[:, b + 3:b + 3 + 2 * jw:2]
                        u = [ut[:, (v * n_ci + q) * 256:(v * n_ci + q) * 256 + jw]
                             for v in range(4)]
                        nc.vector.tensor_tensor(out=u[0], in0=d0, in1=d2, op=SUB)
                        nc.vector.tensor_tensor(out=u[1], in0=d1, in1=d2, op=ADD)
                        nc.vector.tensor_tensor(out=u[2], in0=d2, in1=d1, op=SUB)
                        nc.vector.tensor_tensor(out=u[3], in0=d1, in1=d3, op=SUB)
                    for m in range(4):
                        ms = []
                        for v in range(4):
                            ps = pp.tile([128, 512], F32, tag="ps", name="ps")
                            for q in range(n_ci):
                                lhs = wt[:, ((v * n_ci + q) * 4 + m) * 128:
                                         ((v * n_ci + q) * 4 + m) * 128 + 128]
                                nc.tensor.matmul(
                                    out=ps[:, 0:jw],
                                    lhsT=lhs,
                                    rhs=ut[:, (v * n_ci + q) * 256:
                                            (v * n_ci + q) * 256 + jw],
                                    start=(q == 0), stop=(q == n_ci - 1))
                            ms.append(ps)
                        # y_even = m0+m1+m2 ; y_odd = m1-m2-m3.  A DVE op may
                        # read only ONE input from PSUM, so m1/m2 (used twice
                        # each) are copied to SBUF first.
                        c1 = tp.tile([128, 256], BF16, tag="c1", name="c1", bufs=2)
                        c2 = tp.tile([128, 256], BF16, tag="c2", name="c2", bufs=2)
                        te = tp.tile([128, 256], BF16, tag="te", name="te", bufs=2)
                        to = tp.tile([128, 256], BF16, tag="to", name="to", bufs=2)
                        nc.vector.tensor_scalar_add(c1[:, 0:jw], ms[1][:, 0:jw], 0.0)
                        nc.vector.tensor_scalar_add(c2[:, 0:jw], ms[2][:, 0:jw], 0.0)
                        nc.vector.tensor_tensor(out=te[:, 0:jw], in0=ms[0][:, 0:jw],
                                                in1=c1[:, 0:jw], op=ADD)
                        nc.vector.tensor_tensor(out=te[:, 0:jw], in0=te[:, 0:jw],
                                                in1=c2[:, 0:jw], op=ADD)
                        nc.vector.tensor_tensor(out=to[:, 0:jw], in0=c1[:, 0:jw],
                                                in1=c2[:, 0:jw], op=SUB)
                        nc.vector.tensor_tensor(out=to[:, 0:jw], in0=to[:, 0:jw],
                                                in1=ms[3][:, 0:jw], op=SUB)
                        yb = m * Wq + 1 + 2 * j0
                        if residual is None:
                            nc.scalar.activation(
                                out=Y[:, yb:yb + 2 * jw:2], in_=te[:, 0:jw],
                                func=RELU, bias=bt[:, m:m + 1], scale=1.0)
                            nc.scalar.activation(
                                out=Y[:, yb + 1:yb + 1 + 2 * jw:2], in_=to[:, 0:jw],
                                func=RELU, bias=bt[:, m:m + 1], scale=1.0)
                        else:
                            ee = tp.tile([128, 256], BF16, tag="ee", name="ee", bufs=2)
                            eo = tp.tile([128, 256], BF16, tag="eo", name="eo", bufs=2)
                            nc.scalar.activation(out=ee[:, 0:jw], in_=te[:, 0:jw],
                                                 func=RELU, bias=bt[:, m:m + 1],
                                                 scale=1.0)
                            nc.scalar.activation(out=eo[:, 0:jw], in_=to[:, 0:jw],
                                                 func=RELU, bias=bt[:, m:m + 1],
                                                 scale=1.0)
                            nc.vector.tensor_tensor(
                                out=Y[:, yb:yb + 2 * jw:2], in0=ee[:, 0:jw],
                                in1=residual[:, yb:yb + 2 * jw:2], op=ADD)
                            nc.vector.tensor_tensor(
                                out=Y[:, yb + 1:yb + 1 + 2 * jw:2], in0=eo[:, 0:jw],
                                in1=residual[:, yb + 1:yb + 1 + 2 * jw:2], op=ADD)

            def highway_layer(X, Y, bl):
                """Y = g*relu(h) + (1-g)*X over the strip; X,Y act buffers."""
                for lo, cw in CH:
                    pss = []
                    for m in range(8):
                        ps = pp.tile([128, 512], F32, tag="ps", name="ps")
                        for q in range(4):
                            base = (bl * 4 + q) * 1024 + m * 128
                            nc.tensor.matmul(
                                out=ps[:, 0:cw], lhsT=t_whw[:, base:base + 128],
                                rhs=X[:, q * Wq + 1 + lo:q * Wq + 1 + lo + cw],
                                start=(q == 0), stop=(q == 3))
                        pss.append(ps)
                    for c in range(4):
                        xs = X[:, c * Wq + 1 + lo:c * Wq + 1 + lo + cw]
                        h_t = tp.tile([128, 512], BF16, tag="h", name="h_t", bufs=2)
                        g_t = tp.tile([128, 512], BF16, tag="g", name="g_t", bufs=2)
                        d_t = tp.tile([128, 512], BF16, tag="d", name="d_t", bufs=2)
                        nc.scalar.activation(out=h_t[:, 0:cw], in_=pss[c][:, 0:cw],
                                             func=RELU,
                                             bias=t_bhw[:, bl * 8 + c:bl * 8 + c + 1],
                                             scale=1.0)
                        nc.scalar.activation(out=g_t[:, 0:cw], in_=pss[4 + c][:, 0:cw],
                                             func=SIGM,
                                             bias=t_bhw[:, bl * 8 + 4 + c:bl * 8 + 4 + c + 1],
                                             scale=1.0)
                        nc.vector.tensor_tensor(out=d_t[:, 0:cw], in0=h_t[:, 0:cw],
                                                in1=xs, op=SUB)
                        nc.vector.tensor_tensor(out=d_t[:, 0:cw], in0=d_t[:, 0:cw],
                                                in1=g_t[:, 0:cw], op=MUL)
                        nc.vector.tensor_tensor(
                            out=Y[:, c * Wq + 1 + lo:c * Wq + 1 + lo + cw],
                            in0=d_t[:, 0:cw], in1=xs, op=ADD)

            scope = nc.named_scope

            # ---------- conv0 + highway block 0 ----------
            with scope("conv0"):
                x1 = act_buf("actA")
                wino_conv(t_x0, lambda q: 0, x1, t_wc0, t_bc0, 1)
            with scope("hw0l0"):
                x1b = act_buf("actB")
                highway_layer(x1, x1b, 0)
            with scope("hw0l1"):
                x1c = act_buf("actC")
                highway_layer(x1b, x1c, 1)
                # force SAME-pad zeros at each region's Z col for conv1
                for q in range(4):
                    for s in range(2):
                        z = q * Wq + 1 + s * R
                        nc.vector.memset(x1c[:, z:z + 1], 0)

            # ---------- conv1 (+res) + highway block 1 ----------
            with scope("conv1"):
                x2p = act_buf("actA")
                wino_conv(x1c, lambda q: q * Wq, x2p, t_wc1, t_bc1, 4,
                          residual=x1c)
            with scope("hw1l0"):
                x2b = act_buf("actB")
                highway_layer(x2p, x2b, 2)
            with scope("hw1l1"):
                x2 = act_buf("actC")
                highway_layer(x2b, x2, 3)

            # ---------- ragged word max pool + projection ----------
            with scope("poolproj"):
                t_am = apool.tile([128, 2 * W], BF16, tag="actA", name="t_am")
                nc.scalar.dma_start(out=t_am[:], in_=a_msk[:])
                msel = apool.tile([128, 4 * W], BF16, tag="actB", name="msel")
                for lo, cw in CH:
                    for c in range(4):
                        b = c * Wq + 1
                        s1 = tp.tile([128, 512], BF16, tag="s1", name="s1", bufs=2)
                        s2 = tp.tile([128, 512], BF16, tag="s2", name="s2", bufs=2)
                        nc.vector.tensor_tensor(out=s1[:, 0:cw],
                                                in0=x2[:, b + 1 + lo:b + 1 + lo + cw],
                                                in1=t_am[:, lo:lo + cw], op=ADD)
                        nc.vector.tensor_tensor(out=s2[:, 0:cw],
                                                in0=x2[:, b + 2 + lo:b + 2 + lo + cw],
                                                in1=t_am[:, W + lo:W + lo + cw], op=ADD)
                        nc.vector.tensor_tensor(out=s1[:, 0:cw], in0=s1[:, 0:cw],
                                                in1=s2[:, 0:cw], op=MAX)
                        nc.vector.tensor_tensor(out=msel[:, c * W + lo:c * W + lo + cw],
                                                in0=s1[:, 0:cw],
                                                in1=x2[:, b + lo:b + lo + cw], op=MAX)
                    for m in range(4):
                        ps = pp.tile([128, 512], F32, tag="ps", name="ps")
                        for q in range(4):
                            nc.tensor.matmul(
                                out=ps[:, 0:cw],
                                lhsT=t_wpr[:, q * WED + m * 128:q * WED + (m + 1) * 128],
                                rhs=msel[:, q * W + lo:q * W + lo + cw],
                                start=(q == 0), stop=(q == 3))
                        o_t = tp.tile([128, 512], BF16, tag="o", name="o_t", bufs=3)
                        nc.scalar.activation(out=o_t[:, 0:cw], in_=ps[:, 0:cw],
                                             func=IDEN, bias=t_bpr[:, m:m + 1],
                                             scale=1.0)
                        nc.sync.dma_start(out=out[m * 128:(m + 1) * 128, lo:lo + cw],
                                          in_=o_t[:, 0:cw])

    nc.compile()
    return nc


def _layout(pool_lengths):
    src = pool_lengths.sum(axis=1).astype(np.int64)
    cmax = int(src.max())
    R = max(2112, -(-int(cmax + 4) // 64) * 64)
    return src, R


def _prep_inputs(inputs):
    """Host-side: shard + convert to the kernel's DRAM tensor layouts."""
    byte_tokens = np.asarray(inputs["byte_tokens"], np.int64)
    bpe_mask = np.asarray(inputs["bpe_mask"], bool)
    pool_lengths = np.asarray(inputs["pool_lengths"], np.int64)
    tok_emb = np.asarray(inputs["tok_emb"], np.float32)
    src, R = _layout(pool_lengths)
    W = 2 * R

    def bf(x):
        return np.ascontiguousarray(np.asarray(x, np.float32).astype(_BF16_NP))

    conv0_W = np.asarray(inputs["conv0_W"], np.float32)   # [3,128,512]
    conv1_W = np.asarray(inputs["conv1_W"], np.float32)   # [3,512,512]
    hw0_W = np.asarray(inputs["hw0_W"], np.float32)       # [2,1024,512]
    hw1_W = np.asarray(inputs["hw1_W"], np.float32)
    proj_W = np.asarray(inputs["proj_W"], np.float32)     # [512,512]

    def wino_weights(Wc, n_ci):
        # variants [G0, Ga, Gb, G3]: [3,Cin,Cout] -> [128, (v*n_ci+q)*4*128+m*128+j]
        g0, g1, g2 = Wc[0], Wc[1], Wc[2]
        var = [g0, (g0 + g1 + g2) * 0.5, (g0 - g1 + g2) * 0.5, g2]  # [Cin,Cout]
        out = np.empty((128, 4 * n_ci * 4 * 128), np.float32)
        for v in range(4):
            for q in range(n_ci):
                blk = var[v][q * 128:(q + 1) * 128]  # [128ci, 512co]
                for m in range(4):
                    col = ((v * n_ci + q) * 4 + m) * 128
                    out[:, col:col + 128] = blk[:, m * 128:(m + 1) * 128]
        return bf(out)

    w_c0 = wino_weights(conv0_W, 1)
    w_c1 = wino_weights(conv1_W, 4)
    whw = np.empty((128, 16, 1024), np.float32)
    for bl, (blk, lay) in enumerate(((hw0_W, 0), (hw0_W, 1), (hw1_W, 0), (hw1_W, 1))):
        wt = blk[lay].T  # [512, 1024]
        for q in range(4):
            whw[:, bl * 4 + q, :] = wt[q * 128:(q + 1) * 128]
    w_hw = bf(whw.reshape(128, 16 * 1024))
    w_pr = bf(proj_W.T.reshape(4, 128, WED).transpose(1, 0, 2).reshape(128, 4 * WED))

    def colchunks(b):  # [512] -> [128, 4]
        return np.ascontiguousarray(np.asarray(b, np.float32).reshape(4, 128).T)

    b_c0 = colchunks(inputs["conv0_b"])
    b_c1 = colchunks(inputs["conv1_b"])
    bhw = np.empty((128, 4, 8), np.float32)
    for bl, (blk, lay) in enumerate((("hw0_b", 0), ("hw0_b", 1), ("hw1_b", 0), ("hw1_b", 1))):
        b = np.asarray(inputs[blk], np.float32)[lay]      # [1024]
        bhw[:, bl, 0:4] = b[:512].reshape(4, 128).T
        bhw[:, bl, 4:8] = b[512:1024].reshape(4, 128).T
    b_hw = np.ascontiguousarray(bhw.reshape(128, 32))
    b_pr = colchunks(inputs["proj_b"])

    shared = dict(w_c0=w_c0, w_c1=w_c1, w_hw=w_hw, w_pr=w_pr,
                  b_c0=b_c0, b_c1=b_c1, b_hw=b_hw, b_pr=b_pr)

    emb4 = tok_emb[BPE_MASK_IDX]
    in_maps = []
    meta = []
    for core in range(N_CORES):
        m = dict(shared)
        x0 = np.zeros((128, W), np.float32)
        amsk = np.full((2 * W,), NEG_BIG, np.float32)
        for s in range(SEQ_PER_CORE):
            b = core * SEQ_PER_CORE + s
            L = int(src[b])
            off = s * (W // 2) + 1
            e = tok_emb[byte_tokens[b, :L]]          # [L, 128]
            e = e + emb4 * bpe_mask[b, :L, None]
            x0[:, off:off + L] = e.T
            pl = pool_lengths[b]
            cum = np.cumsum(pl)
            s_w = cum - pl
            st = s_w[pl > 1]
            amsk[off + st] = 0.0
            st = s_w[pl > 2]
            amsk[W + off + st] = 0.0
            meta.append((s_w, pl, off))
        m["x0_in"] = x0.astype(_BF16_NP)
        m["a_msk"] = np.ascontiguousarray(
            np.broadcast_to(amsk.astype(_BF16_NP), (128, 2 * W)))
        in_maps.append(m)
    return in_maps, meta, R


def kernel(**inputs) -> np.ndarray:
    from concourse.bass_utils import run_bass_kernel_spmd

    pool_lengths = np.asarray(inputs["pool_lengths"], np.int64)
    _, R = _layout(pool_lengths)
    if ("nc", R) not in _CACHE:
        _CACHE[("nc", R)] = _build_program(R)
    nc = _CACHE[("nc", R)]
    _CACHE["nc"] = nc  # convenience alias for external profiling harnesses

    in_maps, meta, R = _prep_inputs(inputs)
    res = run_bass_kernel_spmd(nc, in_maps, list(range(N_CORES)))

    proj_b = np.asarray(inputs["proj_b"], np.float32)
    full = np.empty((BSZ, NW, WED), np.float32)
    for core in range(N_CORES):
        o = np.asarray(res.results[core]["out"], np.float32)  # [512, W]
        for s in range(SEQ_PER_CORE):
            b = core * SEQ_PER_CORE + s
            s_w, pl, off = meta[b]
            full[b] = o[:, off + s_w].T
            if (pl == 0).any():
                full[b][pl == 0] = proj_b
    return full


# revision 18
# speedup vs baseline: 1.2853x; 1.1137x over previous
"""Trainium2 Bass kernel for nn_ByteSequenceEmbedder.

Data-parallel across 8 NeuronCores: 2 sequences per core, weights replicated.

v3: both sequences are packed into ONE column strip of W = 2*R columns
(R = 2112 >= max src_len 2085 + halos) and all activations are stored in
EVEN/ODD SPLIT-PLANE order: buffer cols [E0..E_{J-1} | O0..O_{J-1}]
(J = W/2) hold strip cols [0,2,4,..] then [1,3,5,..].  Highway layers and
matmuls are column-order agnostic; the k=3 convs use Winograd F(2,3)
whose input transforms and per-parity outputs become CONTIGUOUS +-1
column shifts inside the planes (full-rate DVE, no strided access):
  u0=O[j-1]-O[j]  u1=E[j]+O[j]  u2=O[j]-E[j]  u3=E[j]-E[j+1]
  m_v = G_v @ u_v (shared PSUM m-terms, 2/3 of direct-conv MACs)
  E_out[j]=m0+m1+m2  O_out[j]=m1-m2-m3
The embedding lookup is precomputed host-side (same DMA bytes as
shipping broadcast tokens).  Ragged word max-pool via host-built
additive masks, also split per parity (x[t+1], x[t+2] become same/+1
col reads in the planes).  Host selects word-start columns from the
[512, W] split-order output.

Per-co-chunk act-buffer layout ("Wq" = W+4 cols):
  [Z | E_0..E_{J-1} | Z | O_0..O_{J-1} | Z | pad]
the middle Z is shared: it is even's right halo (strip col W) and odd's
left halo (strip col -1).  Region Z cols (strip 0 and R -> even j=0 and
j=R/2) are forced zero in conv1's input so each packed sequence sees
SAME-padding.
"""
import numpy as np

import concourse.bacc as bacc
import concourse.tile as tile
import concourse.mybir as mybir

BSZ, NW, T = 16, 1024, 3072
BED, WED = 128, 512
BPE_MASK_IDX = 4
N_CORES = 8
SEQ_PER_CORE = BSZ // N_CORES
BF16 = mybir.dt.bfloat16
F32 = mybir.dt.float32

_BF16_NP = mybir.dt.np(BF16)
NEG_BIG = -1e30

_CACHE = {}


def _chunks(total, step):
    out = []
    lo = 0
    while lo < total:
        out.append((lo, min(step, total - lo)))
        lo += min(step, total - lo)
    return out


def _build_program(R):
    W = 2 * R
    J = W // 2
    Wq = W + 4        # per-co-chunk act buffer width (planes + 3 Z + pad)
    JCH = _chunks(J, 512)   # plane chunks (convs, highway, pool)

    nc = bacc.Bacc("TRN2", target_bir_lowering=False, debug=False)

    def dram_in(name, shape, dt):
        return nc.dram_tensor(name, shape, dt, kind="ExternalInput").ap()

    # winograd conv weights: [128ci, ((v*nci+q)*4+m)*128+j], v in {G0,Ga,Gb,G3}
    w_c0 = dram_in("w_c0", [128, 4 * 1 * WED], BF16)
    w_c1 = dram_in("w_c1", [128, 4 * 4 * WED], BF16)
    w_hw = dram_in("w_hw", [128, 4 * 4 * 1024], BF16)   # [(bl*4+q)*1024 + co]
    w_pr = dram_in("w_pr", [128, 4 * WED], BF16)
    b_c0 = dram_in("b_c0", [128, 4], F32)
    b_c1 = dram_in("b_c1", [128, 4], F32)
    b_hw = dram_in("b_hw", [128, 4 * 8], F32)           # [bl*8 + m]
    b_pr = dram_in("b_pr", [128, 4], F32)
    x0_in = dram_in("x0_in", [128, W], BF16)            # split-order embedding
    a_msk = dram_in("a_msk", [128, 2 * W], BF16)        # [a1e, a1o, a2e, a2o]

    out = nc.dram_tensor("out", [WED, W], BF16, kind="ExternalOutput").ap()

    RELU = mybir.ActivationFunctionType.Relu
    SIGM = mybir.ActivationFunctionType.Sigmoid
    IDEN = mybir.ActivationFunctionType.Identity
    MAX = mybir.AluOpType.max
    ADD = mybir.AluOpType.add
    SUB = mybir.AluOpType.subtract
    MUL = mybir.AluOpType.mult

    with tile.TileContext(nc) as tc:
        with tc.tile_pool(name="wp", bufs=1) as wp, \
             tc.tile_pool(name="ap", bufs=1) as apool, \
             tc.tile_pool(name="tp", bufs=3) as tp, \
             tc.tile_pool(name="up", bufs=2) as upool, \
             tc.tile_pool(name="pp", bufs=8, space="PSUM") as pp:

            # ---- HAM warm-up: PE activity from t~0 so real matmuls start fast ----
            wu = wp.tile([128, 512], BF16)
            nc.vector.memset(wu[:], 0)
            for _ in range(20):
                wps = pp.tile([128, 512], F32, tag="ps", name="wps")
                nc.tensor.matmul(out=wps[:], lhsT=wu[:, 0:128], rhs=wu[:],
                                 start=True, stop=True)

            # ---- loads: conv0 weights/biases first, hw weights per-layer ----
            t_bc0 = wp.tile([128, 4], F32)
            t_bc1 = wp.tile([128, 4], F32)
            t_bhw = wp.tile([128, 4 * 8], F32)
            t_bpr = wp.tile([128, 4], F32)
            t_wc0 = wp.tile([128, 4 * WED], BF16)
            t_wc1 = wp.tile([128, 4 * 4 * WED], BF16)
            # double-buffered per-layer highway weights (4 KB/partition each)
            t_whw = [wp.tile([128, 4 * 1024], BF16, name=f"t_whw{i}")
                     for i in range(2)]
            t_wpr = wp.tile([128, 4 * WED], BF16)

            def load_hw_layer(bl):
                nc.sync.dma_start(out=t_whw[bl % 2][:],
                                  in_=w_hw[:, bl * 4096:(bl + 1) * 4096])

            # x0 strip, split planes, with Z cols; content E at 1..J, O at J+2..
            t_x0 = apool.tile([128, Wq], BF16, tag="actC", name="t_x0")
            for z in (0, J + 1, 2 * J + 2, 2 * J + 3):
                nc.vector.memset(t_x0[:, z:z + 1], 0)
            for p in range(2):
                for lo, cw in JCH:
                    nc.scalar.dma_start(
                        out=t_x0[:, 1 + p * (J + 1) + lo:1 + p * (J + 1) + lo + cw],
                        in_=x0_in[:, p * J + lo:p * J + lo + cw])
            for t, d in ((t_bc0, b_c0), (t_wc0, w_c0), (t_bc1, b_c1),
                         (t_bhw, b_hw), (t_bpr, b_pr)):
                nc.sync.dma_start(out=t[:], in_=d[:])
            load_hw_layer(0)
            load_hw_layer(1)
            nc.gpsimd.dma_start(out=t_wc1[:], in_=w_c1[:])
            nc.gpsimd.dma_start(out=t_wpr[:], in_=w_pr[:])

            def act_buf(tag):
                b = apool.tile([128, 4 * Wq], BF16, tag=tag, name=tag)
                for q in range(4):
                    for z in (0, J + 1, 2 * J + 2, 2 * J + 3):
                        nc.vector.memset(b[:, q * Wq + z:q * Wq + z + 1], 0)
                return b

            def EB(q):  # even-plane content base
                return q * Wq + 1

            def OB(q):  # odd-plane content base
                return q * Wq + J + 2

            def wino_conv(X, xq, Y, wt, bt, n_ci, residual=None):
                """Y = relu(conv3(X)+b) (+residual) in split-plane layout."""
                for j0, jw in JCH:
                    ut = upool.tile([128, 4 * n_ci * 512], BF16, tag="u", name="ut")
                    for q in range(n_ci):
                        E = xq(q) + 1
                        O = xq(q) + J + 2
                        ej = X[:, E + j0:E + j0 + jw]
                        ej1 = X[:, E + j0 + 1:E + j0 + 1 + jw]
                        oj = X[:, O + j0:O + j0 + jw]
                        ojm = X[:, O + j0 - 1:O + j0 - 1 + jw]
                        u = [ut[:, (v * n_ci + q) * 512:(v * n_ci + q) * 512 + jw]
                             for v in range(4)]
                        nc.vector.tensor_tensor(out=u[0], in0=ojm, in1=oj, op=SUB)
                        nc.vector.tensor_tensor(out=u[1], in0=ej, in1=oj, op=ADD)
                        nc.vector.tensor_tensor(out=u[2], in0=oj, in1=ej, op=SUB)
                        nc.vector.tensor_tensor(out=u[3], in0=ej, in1=ej1, op=SUB)
                    for m in range(4):
                        ms = []
                        for v in range(4):
                            ps = pp.tile([128, 512], F32, tag="ps", name="ps")
                            for q in range(n_ci):
                                lhs = wt[:, ((v * n_ci + q) * 4 + m) * 128:
                                         ((v * n_ci + q) * 4 + m) * 128 + 128]
                                nc.tensor.matmul(
                                    out=ps[:, 0:jw],
                                    lhsT=lhs,
                                    rhs=ut[:, (v * n_ci + q) * 512:
                                            (v * n_ci + q) * 512 + jw],
                                    start=(q == 0), stop=(q == n_ci - 1))
                            ms.append(ps)
                        # E_out = m0+m1+m2 ; O_out = m1-m2-m3.  A DVE op may
                        # read only ONE PSUM input, so m1/m2 (used twice) are
                        # copied to SBUF first.
                        c1 = tp.tile([128, 512], BF16, tag="c1", name="c1", bufs=2)
                        c2 = tp.tile([128, 512], BF16, tag="c2", name="c2", bufs=2)
                        te = tp.tile([128, 512], BF16, tag="te", name="te", bufs=2)
                        to = tp.tile([128, 512], BF16, tag="to", name="to", bufs=2)
                        nc.vector.tensor_scalar_add(c1[:, 0:jw], ms[1][:, 0:jw], 0.0)
                        nc.vector.tensor_scalar_add(c2[:, 0:jw], ms[2][:, 0:jw], 0.0)
                        nc.vector.tensor_tensor(out=te[:, 0:jw], in0=ms[0][:, 0:jw],
                                                in1=c1[:, 0:jw], op=ADD)
                        nc.vector.tensor_tensor(out=te[:, 0:jw], in0=te[:, 0:jw],
                                                in1=c2[:, 0:jw], op=ADD)
                        nc.vector.tensor_tensor(out=to[:, 0:jw], in0=c1[:, 0:jw],
                                                in1=c2[:, 0:jw], op=SUB)
                        nc.vector.tensor_tensor(out=to[:, 0:jw], in0=to[:, 0:jw],
                                                in1=ms[3][:, 0:jw], op=SUB)
                        ye = Y[:, EB(m) + j0:EB(m) + j0 + jw]
                        yo = Y[:, OB(m) + j0:OB(m) + j0 + jw]
                        if residual is None:
                            nc.scalar.activation(out=ye, in_=te[:, 0:jw], func=RELU,
                                                 bias=bt[:, m:m + 1], scale=1.0)
                            nc.scalar.activation(out=yo, in_=to[:, 0:jw], func=RELU,
                                                 bias=bt[:, m:m + 1], scale=1.0)
                        else:
                            ee = tp.tile([128, 512], BF16, tag="ee", name="ee", bufs=2)
                            eo = tp.tile([128, 512], BF16, tag="eo", name="eo", bufs=2)
                            nc.scalar.activation(out=ee[:, 0:jw], in_=te[:, 0:jw],
                                                 func=RELU, bias=bt[:, m:m + 1],
                                                 scale=1.0)
                            nc.scalar.activation(out=eo[:, 0:jw], in_=to[:, 0:jw],
                                                 func=RELU, bias=bt[:, m:m + 1],
                                                 scale=1.0)
                            nc.vector.tensor_tensor(
                                out=ye, in0=ee[:, 0:jw],
                                in1=residual[:, EB(m) + j0:EB(m) + j0 + jw], op=ADD)
                            nc.vector.tensor_tensor(
                                out=yo, in0=eo[:, 0:jw],
                                in1=residual[:, OB(m) + j0:OB(m) + j0 + jw], op=ADD)

            def highway_layer(X, Y, bl):
                """Y = g*relu(h) + (1-g)*X per plane chunk."""
                whw = t_whw[bl % 2]
                for p in range(2):
                    for lo, cw in JCH:
                        xb = 1 + p * (J + 1) + lo
                        pss = []
                        for m in range(8):
                            ps = pp.tile([128, 512], F32, tag="ps", name="ps")
                            for q in range(4):
                                base = q * 1024 + m * 128
                                nc.tensor.matmul(
                                    out=ps[:, 0:cw], lhsT=whw[:, base:base + 128],
                                    rhs=X[:, q * Wq + xb:q * Wq + xb + cw],
                                    start=(q == 0), stop=(q == 3))
                            pss.append(ps)
                        for c in range(4):
                            xs = X[:, c * Wq + xb:c * Wq + xb + cw]
                            h_t = tp.tile([128, 512], BF16, tag="h", name="h_t", bufs=2)
                            g_t = tp.tile([128, 512], BF16, tag="g", name="g_t", bufs=2)
                            d_t = tp.tile([128, 512], BF16, tag="d", name="d_t", bufs=2)
                            nc.scalar.activation(
                                out=h_t[:, 0:cw], in_=pss[c][:, 0:cw], func=RELU,
                                bias=t_bhw[:, bl * 8 + c:bl * 8 + c + 1], scale=1.0)
                            nc.scalar.activation(
                                out=g_t[:, 0:cw], in_=pss[4 + c][:, 0:cw], func=SIGM,
                                bias=t_bhw[:, bl * 8 + 4 + c:bl * 8 + 4 + c + 1],
                                scale=1.0)
                            nc.vector.tensor_tensor(out=d_t[:, 0:cw], in0=h_t[:, 0:cw],
                                                    in1=xs, op=SUB)
                            nc.vector.tensor_tensor(out=d_t[:, 0:cw], in0=d_t[:, 0:cw],
                                                    in1=g_t[:, 0:cw], op=MUL)
                            nc.vector.tensor_tensor(
                                out=Y[:, c * Wq + xb:c * Wq + xb + cw],
                                in0=d_t[:, 0:cw], in1=xs, op=ADD)

            scope = nc.named_scope

            # ---------- conv0 + highway block 0 ----------
            with scope("conv0"):
                x1 = act_buf("actA")
                wino_conv(t_x0, lambda q: 0, x1, t_wc0, t_bc0, 1)
            with scope("hw0l0"):
                x1b = act_buf("actB")
                highway_layer(x1, x1b, 0)
            load_hw_layer(2)
            with scope("hw0l1"):
                x1c = act_buf("actC")
                highway_layer(x1b, x1c, 1)
                # force SAME-pad zeros at region Z cols (strip 0 and R ->
                # even-plane j = 0 and R/2) before conv1 reads them
                for q in range(4):
                    for s in range(2):
                        z = EB(q) + s * (R // 2)
                        nc.vector.memset(x1c[:, z:z + 1], 0)
            load_hw_layer(3)

            # ---------- conv1 (+res) + highway block 1 ----------
            with scope("conv1"):
                x2p = act_buf("actA")
                wino_conv(x1c, lambda q: q * Wq, x2p, t_wc1, t_bc1, 4,
                          residual=x1c)
            with scope("hw1l0"):
                x2b = act_buf("actB")
                highway_layer(x2p, x2b, 2)
            with scope("hw1l1"):
                x2 = act_buf("actC")
                highway_layer(x2b, x2, 3)

            # ---------- ragged word max pool + projection ----------
            # even t=2j: x[t]=E[j], x[t+1]=O[j],   x[t+2]=E[j+1]
            # odd  t=2j+1:      O[j],      E[j+1],        O[j+1]
            with scope("poolproj"):
                t_am = apool.tile([128, 2 * W], BF16, tag="actA", name="t_am")
                nc.scalar.dma_start(out=t_am[:], in_=a_msk[:])
                msel = apool.tile([128, 4 * W], BF16, tag="actB", name="msel")
                for p in range(2):
                    for lo, cw in JCH:
                        for c in range(4):
                            if p == 0:
                                x_t = x2[:, EB(c) + lo:EB(c) + lo + cw]
                                x_1 = x2[:, OB(c) + lo:OB(c) + lo + cw]
                                x_2 = x2[:, EB(c) + 1 + lo:EB(c) + 1 + lo + cw]
                            else:
                                x_t = x2[:, OB(c) + lo:OB(c) + lo + cw]
                                x_1 = x2[:, EB(c) + 1 + lo:EB(c) + 1 + lo + cw]
                                x_2 = x2[:, OB(c) + 1 + lo:OB(c) + 1 + lo + cw]
                            s1 = tp.tile([128, 512], BF16, tag="s1", name="s1", bufs=2)
                            s2 = tp.tile([128, 512], BF16, tag="s2", name="s2", bufs=2)
                            nc.vector.tensor_tensor(
                                out=s1[:, 0:cw], in0=x_1,
                                in1=t_am[:, p * J + lo:p * J + lo + cw], op=ADD)
                            nc.vector.tensor_tensor(
                                out=s2[:, 0:cw], in0=x_2,
                                in1=t_am[:, W + p * J + lo:W + p * J + lo + cw],
                                op=ADD)
                            nc.vector.tensor_tensor(out=s1[:, 0:cw], in0=s1[:, 0:cw],
                                                    in1=s2[:, 0:cw], op=MAX)
                            nc.vector.tensor_tensor(
                                out=msel[:, c * W + p * J + lo:c * W + p * J + lo + cw],
                                in0=s1[:, 0:cw], in1=x_t, op=MAX)
                        for m in range(4):
                            ps = pp.tile([128, 512], F32, tag="ps", name="ps")
                            for q in range(4):
                                nc.tensor.matmul(
                                    out=ps[:, 0:cw],
                                    lhsT=t_wpr[:, q * WED + m * 128:q * WED + (m + 1) * 128],
                                    rhs=msel[:, q * W + p * J + lo:q * W + p * J + lo + cw],
                                    start=(q == 0), stop=(q == 3))
                            o_t = tp.tile([128, 512], BF16, tag="o", name="o_t", bufs=3)
                            nc.scalar.activation(out=o_t[:, 0:cw], in_=ps[:, 0:cw],
                                                 func=IDEN, bias=t_bpr[:, m:m + 1],
                                                 scale=1.0)
                            nc.sync.dma_start(
                                out=out[m * 128:(m + 1) * 128,
                                        p * J + lo:p * J + lo + cw],
                                in_=o_t[:, 0:cw])

    nc.compile()
    return nc


def _layout(pool_lengths):
    src = pool_lengths.sum(axis=1).astype(np.int64)
    cmax = int(src.max())
    R = max(2112, -(-int(cmax + 4) // 64) * 64)
    return src, R


def _prep_inputs(inputs):
    """Host-side: shard + convert to the kernel's DRAM tensor layouts."""
    byte_tokens = np.asarray(inputs["byte_tokens"], np.int64)
    bpe_mask = np.asarray(inputs["bpe_mask"], bool)
    pool_lengths = np.asarray(inputs["pool_lengths"], np.int64)
    tok_emb = np.asarray(inputs["tok_emb"], np.float32)
    src, R = _layout(pool_lengths)
    W = 2 * R
    J = W // 2

    def bf(x):
        return np.ascontiguousarray(np.asarray(x, np.float32).astype(_BF16_NP))

    conv0_W = np.asarray(inputs["conv0_W"], np.float32)   # [3,128,512]
    conv1_W = np.asarray(inputs["conv1_W"], np.float32)   # [3,512,512]
    hw0_W = np.asarray(inputs["hw0_W"], np.float32)       # [2,1024,512]
    hw1_W = np.asarray(inputs["hw1_W"], np.float32)
    proj_W = np.asarray(inputs["proj_W"], np.float32)     # [512,512]

    def wino_weights(Wc, n_ci):
        # variants [G0, Ga, Gb, G3]: [3,Cin,Cout] -> [128, (v*n_ci+q)*4*128+m*128+j]
        g0, g1, g2 = Wc[0], Wc[1], Wc[2]
        var = [g0, (g0 + g1 + g2) * 0.5, (g0 - g1 + g2) * 0.5, g2]  # [Cin,Cout]
        out = np.empty((128, 4 * n_ci * 4 * 128), np.float32)
        for v in range(4):
            for q in range(n_ci):
                blk = var[v][q * 128:(q + 1) * 128]  # [128ci, 512co]
                for m in range(4):
                    col = ((v * n_ci + q) * 4 + m) * 128
                    out[:, col:col + 128] = blk[:, m * 128:(m + 1) * 128]
        return bf(out)

    w_c0 = wino_weights(conv0_W, 1)
    w_c1 = wino_weights(conv1_W, 4)
    whw = np.empty((128, 16, 1024), np.float32)
    for bl, (blk, lay) in enumerate(((hw0_W, 0), (hw0_W, 1), (hw1_W, 0), (hw1_W, 1))):
        wt = blk[lay].T  # [512, 1024]
        for q in range(4):
            whw[:, bl * 4 + q, :] = wt[q * 128:(q + 1) * 128]
    w_hw = bf(whw.reshape(128, 16 * 1024))
    w_pr = bf(proj_W.T.reshape(4, 128, WED).transpose(1, 0, 2).reshape(128, 4 * WED))

    def colchunks(b):  # [512] -> [128, 4]
        return np.ascontiguousarray(np.asarray(b, np.float32).reshape(4, 128).T)

    b_c0 = colchunks(inputs["conv0_b"])
    b_c1 = colchunks(inputs["conv1_b"])
    bhw = np.empty((128, 4, 8), np.float32)
    for bl, (blk, lay) in enumerate((("hw0_b", 0), ("hw0_b", 1), ("hw1_b", 0), ("hw1_b", 1))):
        b = np.asarray(inputs[blk], np.float32)[lay]      # [1024]
        bhw[:, bl, 0:4] = b[:512].reshape(4, 128).T
        bhw[:, bl, 4:8] = b[512:1024].reshape(4, 128).T
    b_hw = np.ascontiguousarray(bhw.reshape(128, 32))
    b_pr = colchunks(inputs["proj_b"])

    shared = dict(w_c0=w_c0, w_c1=w_c1, w_hw=w_hw, w_pr=w_pr,
                  b_c0=b_c0, b_c1=b_c1, b_hw=b_hw, b_pr=b_pr)

    emb4 = tok_emb[BPE_MASK_IDX]
    in_maps = []
    meta = []
    for core in range(N_CORES):
        m = dict(shared)
        x0 = np.zeros((128, W), np.float32)      # strip order first
        a1 = np.full((W,), NEG_BIG, np.float32)
        a2 = np.full((W,), NEG_BIG, np.float32)
        for s in range(SEQ_PER_CORE):
            b = core * SEQ_PER_CORE + s
            L = int(src[b])
            off = s * R + 1
            e = tok_emb[byte_tokens[b, :L]]          # [L, 128]
            e = e + emb4 * bpe_mask[b, :L, None]
            x0[:, off:off + L] = e.T
            pl = pool_lengths[b]
            cum = np.cumsum(pl)
            s_w = cum - pl
            st = s_w[pl > 1]
            a1[off + st] = 0.0
            st = s_w[pl > 2]
            a2[off + st] = 0.0
            meta.append((s_w, pl, off))
        # split to even/odd planes
        x0s = np.concatenate([x0[:, 0::2], x0[:, 1::2]], axis=1)
        amsk = np.concatenate([a1[0::2], a1[1::2], a2[0::2], a2[1::2]])
        m["x0_in"] = x0s.astype(_BF16_NP)
        m["a_msk"] = np.ascontiguousarray(
            np.broadcast_to(amsk.astype(_BF16_NP), (128, 2 * W)))
        in_maps.append(m)
    return in_maps, meta, R


def kernel(**inputs) -> np.ndarray:
    from concourse.bass_utils import run_bass_kernel_spmd

    pool_lengths = np.asarray(inputs["pool_lengths"], np.int64)
    _, R = _layout(pool_lengths)
    if ("nc", R) not in _CACHE:
        _CACHE[("nc", R)] = _build_program(R)
    nc = _CACHE[("nc", R)]
    _CACHE["nc"] = nc  # convenience alias for external profiling harnesses

    in_maps, meta, R = _prep_inputs(inputs)
    res = run_bass_kernel_spmd(nc, in_maps, list(range(N_CORES)))

    J = R  # J = W/2 = R
    proj_b = np.asarray(inputs["proj_b"], np.float32)
    full = np.empty((BSZ, NW, WED), np.float32)
    for core in range(N_CORES):
        o = np.asarray(res.results[core]["out"], np.float32)  # [512, W] split order
        for s in range(SEQ_PER_CORE):
            b = core * SEQ_PER_CORE + s
            s_w, pl, off = meta[b]
            c = off + s_w                         # strip cols
            cols = np.where(c % 2 == 0, c // 2, J + c // 2)
            full[b] = o[:, cols].T
            if (pl == 0).any():
                full[b][pl == 0] = proj_b
    return full


# revision 27
# speedup vs baseline: 1.3888x; 1.0805x over previous
"""Trainium2 Bass kernel for nn_ByteSequenceEmbedder.

Data-parallel across 8 NeuronCores: 2 sequences per core, weights replicated.

v3: both sequences are packed into ONE column strip of W = 2*R columns
(R = 2112 >= max src_len 2085 + halos) and all activations are stored in
EVEN/ODD SPLIT-PLANE order: buffer cols [E0..E_{J-1} | O0..O_{J-1}]
(J = W/2) hold strip cols [0,2,4,..] then [1,3,5,..].  Highway layers and
matmuls are column-order agnostic; the k=3 convs use Winograd F(2,3)
whose input transforms and per-parity outputs become CONTIGUOUS +-1
column shifts inside the planes (full-rate DVE, no strided access):
  u0=O[j-1]-O[j]  u1=E[j]+O[j]  u2=O[j]-E[j]  u3=E[j]-E[j+1]
  m_v = G_v @ u_v (shared PSUM m-terms, 2/3 of direct-conv MACs)
  E_out[j]=m0+m1+m2  O_out[j]=m1-m2-m3
The embedding lookup is precomputed host-side (same DMA bytes as
shipping broadcast tokens).  Ragged word max-pool via host-built
additive masks, also split per parity (x[t+1], x[t+2] become same/+1
col reads in the planes).  Host selects word-start columns from the
[512, W] split-order output.

Per-co-chunk act-buffer layout ("Wq" = W+4 cols):
  [Z | E_0..E_{J-1} | Z | O_0..O_{J-1} | Z | pad]
the middle Z is shared: it is even's right halo (strip col W) and odd's
left halo (strip col -1).  Region Z cols (strip 0 and R -> even j=0 and
j=R/2) are forced zero in conv1's input so each packed sequence sees
SAME-padding.
"""
import numpy as np

import concourse.bacc as bacc
import concourse.tile as tile
import concourse.mybir as mybir

BSZ, NW, T = 16, 1024, 3072
BED, WED = 128, 512
BPE_MASK_IDX = 4
N_CORES = 8
SEQ_PER_CORE = BSZ // N_CORES
BF16 = mybir.dt.bfloat16
F32 = mybir.dt.float32

_BF16_NP = mybir.dt.np(BF16)
NEG_BIG = -1e30

_CACHE = {}


def _chunks(total, step):
    out = []
    lo = 0
    while lo < total:
        out.append((lo, min(step, total - lo)))
        lo += min(step, total - lo)
    return out


def _build_program(R):
    W = 2 * R
    J = W // 2
    Wq = W + 4        # per-co-chunk act buffer width (planes + 3 Z + pad)
    JCH = _chunks(J, 512)   # plane chunks (convs, highway, pool)

    nc = bacc.Bacc("TRN2", target_bir_lowering=False, debug=False)

    def dram_in(name, shape, dt):
        return nc.dram_tensor(name, shape, dt, kind="ExternalInput").ap()

    # conv0 weights: no-share winograd, v in {G0, Ga, Gb, -Gb, -G3}
    w_c0 = dram_in("w_c0", [128, 5 * WED], BF16)
    # conv1 weights: shared-m winograd, [128ci, ((v*4+q)*4+m)*128+j], v in {G0,Ga,Gb,G3}
    w_c1 = dram_in("w_c1", [128, 4 * 4 * WED], BF16)
    w_hw = dram_in("w_hw", [128, 4 * 4 * 1024], BF16)   # [(bl*4+q)*1024 + co]
    w_pr = dram_in("w_pr", [128, 4 * WED], BF16)
    b_c0 = dram_in("b_c0", [128, 4], F32)
    b_c1 = dram_in("b_c1", [128, 4], F32)
    b_hw = dram_in("b_hw", [128, 4 * 8], F32)           # [bl*8 + m]
    b_pr = dram_in("b_pr", [128, 4], F32)
    x0_in = dram_in("x0_in", [128, W], BF16)            # split-order embedding
    a_msk = dram_in("a_msk", [128, 2 * W], BF16)        # [a1e, a1o, a2e, a2o]

    out = nc.dram_tensor("out", [WED, W], BF16, kind="ExternalOutput").ap()

    RELU = mybir.ActivationFunctionType.Relu
    SIGM = mybir.ActivationFunctionType.Sigmoid
    IDEN = mybir.ActivationFunctionType.Identity
    MAX = mybir.AluOpType.max
    ADD = mybir.AluOpType.add
    SUB = mybir.AluOpType.subtract
    MUL = mybir.AluOpType.mult

    with tile.TileContext(nc) as tc:
        with tc.tile_pool(name="wp", bufs=1) as wp, \
             tc.tile_pool(name="ap", bufs=1) as apool, \
             tc.tile_pool(name="tp", bufs=3) as tp, \
             tc.tile_pool(name="up", bufs=2) as upool, \
             tc.tile_pool(name="pp", bufs=8, space="PSUM") as pp:

            # ---- HAM warm-up: PE activity from t~0 so real matmuls start fast ----
            wu = wp.tile([128, 512], BF16)
            nc.vector.memset(wu[:], 0)
            for _ in range(20):
                wps = pp.tile([128, 512], F32, tag="ps", name="wps")
                nc.tensor.matmul(out=wps[:], lhsT=wu[:, 0:128], rhs=wu[:],
                                 start=True, stop=True)

            # ---- loads: conv0 weights/biases first, hw weights per-layer ----
            t_bc0 = wp.tile([128, 4], F32)
            t_bc1 = wp.tile([128, 4], F32)
            t_bhw = wp.tile([128, 4 * 8], F32)
            t_bpr = wp.tile([128, 4], F32)
            t_wc0 = wp.tile([128, 5 * WED], BF16)
            t_wc1 = wp.tile([128, 4 * 4 * WED], BF16)
            # double-buffered per-layer highway weights (4 KB/partition each)
            t_whw = [wp.tile([128, 4 * 1024], BF16, name=f"t_whw{i}")
                     for i in range(2)]
            t_wpr = wp.tile([128, 4 * WED], BF16)

            def load_hw_layer(bl):
                nc.sync.dma_start(out=t_whw[bl % 2][:],
                                  in_=w_hw[:, bl * 4096:(bl + 1) * 4096])

            # x0 strip, split planes; E content at 2..J+1, O at J+4..2J+3
            # (even bases keep the DVE's packed-bf16 2x read mode eligible)
            t_x0 = apool.tile([128, Wq], BF16, tag="actC", name="t_x0")
            for z in (0, 1, J + 2, J + 3):
                nc.vector.memset(t_x0[:, z:z + 1], 0)
            for lo, cw in JCH:
                for p in range(2):
                    nc.scalar.dma_start(
                        out=t_x0[:, 2 + p * (J + 2) + lo:2 + p * (J + 2) + lo + cw],
                        in_=x0_in[:, p * J + lo:p * J + lo + cw])
            for t, d in ((t_bc0, b_c0), (t_wc0, w_c0), (t_bc1, b_c1),
                         (t_bhw, b_hw), (t_bpr, b_pr)):
                nc.sync.dma_start(out=t[:], in_=d[:])
            load_hw_layer(0)
            load_hw_layer(1)
            nc.gpsimd.dma_start(out=t_wc1[:], in_=w_c1[:])
            nc.gpsimd.dma_start(out=t_wpr[:], in_=w_pr[:])

            def act_buf(tag):
                b = apool.tile([128, 4 * Wq + 2], BF16, tag=tag, name=tag)
                for q in range(4):
                    for z in (0, 1, J + 2, J + 3):
                        nc.vector.memset(b[:, q * Wq + z:q * Wq + z + 1], 0)
                nc.vector.memset(b[:, 4 * Wq:4 * Wq + 2], 0)
                return b

            def EB(q):  # even-plane content base (even offset)
                return q * Wq + 2

            def OB(q):  # odd-plane content base (even offset)
                return q * Wq + J + 4

            def transforms(X, xq, ut, j0, jw, n_ci):
                for q in range(n_ci):
                    E = xq(q) + 2
                    O = xq(q) + J + 4
                    ej = X[:, E + j0:E + j0 + jw]
                    ej1 = X[:, E + j0 + 1:E + j0 + 1 + jw]
                    oj = X[:, O + j0:O + j0 + jw]
                    ojm = X[:, O + j0 - 1:O + j0 - 1 + jw]
                    u = [ut[:, (v * n_ci + q) * 512:(v * n_ci + q) * 512 + jw]
                         for v in range(4)]
                    nc.vector.tensor_tensor(out=u[0], in0=ojm, in1=oj, op=SUB)
                    nc.vector.tensor_tensor(out=u[1], in0=ej, in1=oj, op=ADD)
                    nc.vector.tensor_tensor(out=u[2], in0=oj, in1=ej, op=SUB)
                    nc.vector.tensor_tensor(out=u[3], in0=ej, in1=ej1, op=SUB)

            def wino_conv0(X, Y):
                """conv0 (128ci): no-share F(2,3).  E-psum = G0@u0+Ga@u1+Gb@u2,
                O-psum = Ga@u1-Gb@u2-G3@u3 (signs baked into weight slots);
                relu+bias evac on DVE straight from PSUM."""
                for j0, jw in JCH:
                    ut = upool.tile([128, 4 * 512], BF16, tag="u", name="ut")
                    transforms(X, lambda q: 0, ut, j0, jw, 1)
                    for m in range(4):
                        for par, slots in ((0, (0, 1, 2)), (1, (1, 3, 4))):
                            ps = pp.tile([128, 512], F32, tag="ps", name="ps")
                            for i, v in enumerate(slots):
                                uu = (0, 1, 2) if par == 0 else (1, 2, 3)
                                nc.tensor.matmul(
                                    out=ps[:, 0:jw],
                                    lhsT=t_wc0[:, (v * 4 + m) * 128:(v * 4 + m) * 128 + 128],
                                    rhs=ut[:, uu[i] * 512:uu[i] * 512 + jw],
                                    start=(i == 0), stop=(i == 2))
                            yb = (EB(m) if par == 0 else OB(m)) + j0
                            nc.vector.tensor_scalar(
                                out=Y[:, yb:yb + jw], in0=ps[:, 0:jw],
                                scalar1=t_bc0[:, m:m + 1], scalar2=0.0,
                                op0=ADD, op1=MAX)

            def wino_conv1(X, xq, Y, wt, bt, residual, n_ci=4):
                """conv1 (512ci): shared-m F(2,3) + residual."""
                for j0, jw in JCH:
                    ut = upool.tile([128, 4 * n_ci * 512], BF16, tag="u", name="ut")
                    transforms(X, xq, ut, j0, jw, n_ci)
                    for m in range(4):
                        ms = []
                        for v in range(4):
                            ps = pp.tile([128, 512], F32, tag="ps", name="ps")
                            for q in range(n_ci):
                                lhs = wt[:, ((v * n_ci + q) * 4 + m) * 128:
                                         ((v * n_ci + q) * 4 + m) * 128 + 128]
                                nc.tensor.matmul(
                                    out=ps[:, 0:jw],
                                    lhsT=lhs,
                                    rhs=ut[:, (v * n_ci + q) * 512:
                                            (v * n_ci + q) * 512 + jw],
                                    start=(q == 0), stop=(q == n_ci - 1))
                            ms.append(ps)
                        # E_out = m0+m1+m2 ; O_out = m1-m2-m3.  A DVE op may
                        # read only ONE PSUM input, so m1/m2 (used twice) are
                        # copied to SBUF first.
                        c1 = tp.tile([128, 512], BF16, tag="c1", name="c1", bufs=2)
                        c2 = tp.tile([128, 512], BF16, tag="c2", name="c2", bufs=2)
                        te = tp.tile([128, 512], BF16, tag="te", name="te", bufs=2)
                        to = tp.tile([128, 512], BF16, tag="to", name="to", bufs=2)
                        nc.vector.tensor_scalar_add(c1[:, 0:jw], ms[1][:, 0:jw], 0.0)
                        nc.vector.tensor_scalar_add(c2[:, 0:jw], ms[2][:, 0:jw], 0.0)
                        nc.vector.tensor_tensor(out=te[:, 0:jw], in0=ms[0][:, 0:jw],
                                                in1=c1[:, 0:jw], op=ADD)
                        nc.vector.tensor_tensor(out=te[:, 0:jw], in0=te[:, 0:jw],
                                                in1=c2[:, 0:jw], op=ADD)
                        nc.vector.tensor_tensor(out=to[:, 0:jw], in0=c1[:, 0:jw],
                                                in1=c2[:, 0:jw], op=SUB)
                        nc.vector.tensor_tensor(out=to[:, 0:jw], in0=to[:, 0:jw],
                                                in1=ms[3][:, 0:jw], op=SUB)
                        ye = Y[:, EB(m) + j0:EB(m) + j0 + jw]
                        yo = Y[:, OB(m) + j0:OB(m) + j0 + jw]
                        if residual is None:
                            nc.scalar.activation(out=ye, in_=te[:, 0:jw], func=RELU,
                                                 bias=bt[:, m:m + 1], scale=1.0)
                            nc.scalar.activation(out=yo, in_=to[:, 0:jw], func=RELU,
                                                 bias=bt[:, m:m + 1], scale=1.0)
                        else:
                            ee = tp.tile([128, 512], BF16, tag="ee", name="ee", bufs=2)
                            eo = tp.tile([128, 512], BF16, tag="eo", name="eo", bufs=2)
                            nc.scalar.activation(out=ee[:, 0:jw], in_=te[:, 0:jw],
                                                 func=RELU, bias=bt[:, m:m + 1],
                                                 scale=1.0)
                            nc.scalar.activation(out=eo[:, 0:jw], in_=to[:, 0:jw],
                                                 func=RELU, bias=bt[:, m:m + 1],
                                                 scale=1.0)
                            nc.vector.tensor_tensor(
                                out=ye, in0=ee[:, 0:jw],
                                in1=residual[:, EB(m) + j0:EB(m) + j0 + jw], op=ADD)
                            nc.vector.tensor_tensor(
                                out=yo, in0=eo[:, 0:jw],
                                in1=residual[:, OB(m) + j0:OB(m) + j0 + jw], op=ADD)

            def highway_layer(X, Y, bl):
                """Y = g*relu(h) + (1-g)*X per plane chunk."""
                whw = t_whw[bl % 2]
                for p in range(2):
                    for lo, cw in JCH:
                        xb = 2 + p * (J + 2) + lo
                        pss = []
                        for m in range(8):
                            ps = pp.tile([128, 512], F32, tag="ps", name="ps")
                            for q in range(4):
                                base = q * 1024 + m * 128
                                nc.tensor.matmul(
                                    out=ps[:, 0:cw], lhsT=whw[:, base:base + 128],
                                    rhs=X[:, q * Wq + xb:q * Wq + xb + cw],
                                    start=(q == 0), stop=(q == 3))
                            pss.append(ps)
                        for c in range(4):
                            xs = X[:, c * Wq + xb:c * Wq + xb + cw]
                            h_t = tp.tile([128, 512], BF16, tag="h", name="h_t", bufs=2)
                            g_t = tp.tile([128, 512], BF16, tag="g", name="g_t", bufs=2)
                            d_t = tp.tile([128, 512], BF16, tag="d", name="d_t", bufs=2)
                            nc.scalar.activation(
                                out=h_t[:, 0:cw], in_=pss[c][:, 0:cw], func=RELU,
                                bias=t_bhw[:, bl * 8 + c:bl * 8 + c + 1], scale=1.0)
                            nc.scalar.activation(
                                out=g_t[:, 0:cw], in_=pss[4 + c][:, 0:cw], func=SIGM,
                                bias=t_bhw[:, bl * 8 + 4 + c:bl * 8 + 4 + c + 1],
                                scale=1.0)
                            nc.vector.tensor_tensor(out=d_t[:, 0:cw], in0=h_t[:, 0:cw],
                                                    in1=xs, op=SUB)
                            nc.vector.tensor_tensor(out=d_t[:, 0:cw], in0=d_t[:, 0:cw],
                                                    in1=g_t[:, 0:cw], op=MUL)
                            nc.vector.tensor_tensor(
                                out=Y[:, c * Wq + xb:c * Wq + xb + cw],
                                in0=d_t[:, 0:cw], in1=xs, op=ADD)

            scope = nc.named_scope

            # ---------- conv0 + highway block 0 ----------
            with scope("conv0"):
                x1 = act_buf("actA")
                wino_conv0(t_x0, x1)
            with scope("hw0l0"):
                x1b = act_buf("actB")
                highway_layer(x1, x1b, 0)
            load_hw_layer(2)
            with scope("hw0l1"):
                x1c = act_buf("actC")
                highway_layer(x1b, x1c, 1)
                # force SAME-pad zeros at region Z cols (strip 0 and R ->
                # even-plane j = 0 and R/2) before conv1 reads them
                for q in range(4):
                    for s in range(2):
                        z = EB(q) + s * (R // 2)
                        nc.vector.memset(x1c[:, z:z + 1], 0)
            load_hw_layer(3)

            # ---------- conv1 (+res) + highway block 1 ----------
            with scope("conv1"):
                x2p = act_buf("actA")
                wino_conv1(x1c, lambda q: q * Wq, x2p, t_wc1, t_bc1,
                           residual=x1c)
            with scope("hw1l0"):
                x2b = act_buf("actB")
                highway_layer(x2p, x2b, 2)
            with scope("hw1l1"):
                x2 = act_buf("actC")
                highway_layer(x2b, x2, 3)

            # ---------- ragged word max pool + projection ----------
            # even t=2j: x[t]=E[j], x[t+1]=O[j],   x[t+2]=E[j+1]
            # odd  t=2j+1:      O[j],      E[j+1],        O[j+1]
            with scope("poolproj"):
                t_am = apool.tile([128, 2 * W], BF16, tag="actA", name="t_am")
                nc.scalar.dma_start(out=t_am[:], in_=a_msk[:])
                msel = apool.tile([128, 4 * W], BF16, tag="actB", name="msel")
                for p in range(2):
                    for lo, cw in JCH:
                        for c in range(4):
                            if p == 0:
                                x_t = x2[:, EB(c) + lo:EB(c) + lo + cw]
                                x_1 = x2[:, OB(c) + lo:OB(c) + lo + cw]
                                x_2 = x2[:, EB(c) + 1 + lo:EB(c) + 1 + lo + cw]
                            else:
                                x_t = x2[:, OB(c) + lo:OB(c) + lo + cw]
                                x_1 = x2[:, EB(c) + 1 + lo:EB(c) + 1 + lo + cw]
                                x_2 = x2[:, OB(c) + 1 + lo:OB(c) + 1 + lo + cw]
                            s1 = tp.tile([128, 512], BF16, tag="s1", name="s1", bufs=2)
                            s2 = tp.tile([128, 512], BF16, tag="s2", name="s2", bufs=2)
                            nc.vector.tensor_tensor(
                                out=s1[:, 0:cw], in0=x_1,
                                in1=t_am[:, p * J + lo:p * J + lo + cw], op=ADD)
                            nc.vector.tensor_tensor(
                                out=s2[:, 0:cw], in0=x_2,
                                in1=t_am[:, W + p * J + lo:W + p * J + lo + cw],
                                op=ADD)
                            nc.vector.tensor_tensor(out=s1[:, 0:cw], in0=s1[:, 0:cw],
                                                    in1=s2[:, 0:cw], op=MAX)
                            nc.vector.tensor_tensor(
                                out=msel[:, c * W + p * J + lo:c * W + p * J + lo + cw],
                                in0=s1[:, 0:cw], in1=x_t, op=MAX)
                        for m in range(4):
                            ps = pp.tile([128, 512], F32, tag="ps", name="ps")
                            for q in range(4):
                                nc.tensor.matmul(
                                    out=ps[:, 0:cw],
                                    lhsT=t_wpr[:, q * WED + m * 128:q * WED + (m + 1) * 128],
                                    rhs=msel[:, q * W + p * J + lo:q * W + p * J + lo + cw],
                                    start=(q == 0), stop=(q == 3))
                            o_t = tp.tile([128, 512], BF16, tag="o", name="o_t", bufs=3)
                            nc.scalar.activation(out=o_t[:, 0:cw], in_=ps[:, 0:cw],
                                                 func=IDEN, bias=t_bpr[:, m:m + 1],
                                                 scale=1.0)
                            nc.sync.dma_start(
                                out=out[m * 128:(m + 1) * 128,
                                        p * J + lo:p * J + lo + cw],
                                in_=o_t[:, 0:cw])

    nc.compile()
    return nc


def _layout(pool_lengths):
    src = pool_lengths.sum(axis=1).astype(np.int64)
    cmax = int(src.max())
    R = max(2112, -(-int(cmax + 4) // 64) * 64)
    return src, R


def _prep_inputs(inputs):
    """Host-side: shard + convert to the kernel's DRAM tensor layouts."""
    byte_tokens = np.asarray(inputs["byte_tokens"], np.int64)
    bpe_mask = np.asarray(inputs["bpe_mask"], bool)
    pool_lengths = np.asarray(inputs["pool_lengths"], np.int64)
    tok_emb = np.asarray(inputs["tok_emb"], np.float32)
    src, R = _layout(pool_lengths)
    W = 2 * R
    J = W // 2

    def bf(x):
        return np.ascontiguousarray(np.asarray(x, np.float32).astype(_BF16_NP))

    conv0_W = np.asarray(inputs["conv0_W"], np.float32)   # [3,128,512]
    conv1_W = np.asarray(inputs["conv1_W"], np.float32)   # [3,512,512]
    hw0_W = np.asarray(inputs["hw0_W"], np.float32)       # [2,1024,512]
    hw1_W = np.asarray(inputs["hw1_W"], np.float32)
    proj_W = np.asarray(inputs["proj_W"], np.float32)     # [512,512]

    def wino_weights(Wc, n_ci, var):
        # [Cin,Cout] variant list -> [128, (v*n_ci+q)*4*128 + m*128 + j]
        out = np.empty((128, len(var) * n_ci * 4 * 128), np.float32)
        for v in range(len(var)):
            for q in range(n_ci):
                blk = var[v][q * 128:(q + 1) * 128]  # [128ci, 512co]
                for m in range(4):
                    col = ((v * n_ci + q) * 4 + m) * 128
                    out[:, col:col + 128] = blk[:, m * 128:(m + 1) * 128]
        return bf(out)

    def variants(Wc):
        g0, g1, g2 = Wc[0], Wc[1], Wc[2]
        ga = (g0 + g1 + g2) * 0.5
        gb = (g0 - g1 + g2) * 0.5
        return g0, ga, gb, g2

    g0, ga, gb, g2 = variants(conv0_W)
    w_c0 = wino_weights(conv0_W, 1, [g0, ga, gb, -gb, -g2])
    g0, ga, gb, g2 = variants(conv1_W)
    w_c1 = wino_weights(conv1_W, 4, [g0, ga, gb, g2])
    whw = np.empty((128, 16, 1024), np.float32)
    for bl, (blk, lay) in enumerate(((hw0_W, 0), (hw0_W, 1), (hw1_W, 0), (hw1_W, 1))):
        wt = blk[lay].T  # [512, 1024]
        for q in range(4):
            whw[:, bl * 4 + q, :] = wt[q * 128:(q + 1) * 128]
    w_hw = bf(whw.reshape(128, 16 * 1024))
    w_pr = bf(proj_W.T.reshape(4, 128, WED).transpose(1, 0, 2).reshape(128, 4 * WED))

    def colchunks(b):  # [512] -> [128, 4]
        return np.ascontiguousarray(np.asarray(b, np.float32).reshape(4, 128).T)

    b_c0 = colchunks(inputs["conv0_b"])
    b_c1 = colchunks(inputs["conv1_b"])
    bhw = np.empty((128, 4, 8), np.float32)
    for bl, (blk, lay) in enumerate((("hw0_b", 0), ("hw0_b", 1), ("hw1_b", 0), ("hw1_b", 1))):
        b = np.asarray(inputs[blk], np.float32)[lay]      # [1024]
        bhw[:, bl, 0:4] = b[:512].reshape(4, 128).T
        bhw[:, bl, 4:8] = b[512:1024].reshape(4, 128).T
    b_hw = np.ascontiguousarray(bhw.reshape(128, 32))
    b_pr = colchunks(inputs["proj_b"])

    shared = dict(w_c0=w_c0, w_c1=w_c1, w_hw=w_hw, w_pr=w_pr,
                  b_c0=b_c0, b_c1=b_c1, b_hw=b_hw, b_pr=b_pr)

    emb4 = tok_emb[BPE_MASK_IDX]
    in_maps = []
    meta = []
    for core in range(N_CORES):
        m = dict(shared)
        x0 = np.zeros((128, W), np.float32)      # strip order first
        a1 = np.full((W,), NEG_BIG, np.float32)
        a2 = np.full((W,), NEG_BIG, np.float32)
        for s in range(SEQ_PER_CORE):
            b = core * SEQ_PER_CORE + s
            L = int(src[b])
            off = s * R + 1
            e = tok_emb[byte_tokens[b, :L]]          # [L, 128]
            e = e + emb4 * bpe_mask[b, :L, None]
            x0[:, off:off + L] = e.T
            pl = pool_lengths[b]
            cum = np.cumsum(pl)
            s_w = cum - pl
            st = s_w[pl > 1]
            a1[off + st] = 0.0
            st = s_w[pl > 2]
            a2[off + st] = 0.0
            meta.append((s_w, pl, off))
        # split to even/odd planes
        x0s = np.concatenate([x0[:, 0::2], x0[:, 1::2]], axis=1)
        amsk = np.concatenate([a1[0::2], a1[1::2], a2[0::2], a2[1::2]])
        m["x0_in"] = x0s.astype(_BF16_NP)
        m["a_msk"] = np.ascontiguousarray(
            np.broadcast_to(amsk.astype(_BF16_NP), (128, 2 * W)))
        in_maps.append(m)
    return in_maps, meta, R


def kernel(**inputs) -> np.ndarray:
    from concourse.bass_utils import run_bass_kernel_spmd

    pool_lengths = np.asarray(inputs["pool_lengths"], np.int64)
    _, R = _layout(pool_lengths)
    if ("nc", R) not in _CACHE:
        _CACHE[("nc", R)] = _build_program(R)
    nc = _CACHE[("nc", R)]
    _CACHE["nc"] = nc  # convenience alias for external profiling harnesses

    in_maps, meta, R = _prep_inputs(inputs)
    res = run_bass_kernel_spmd(nc, in_maps, list(range(N_CORES)))

    J = R  # J = W/2 = R
    proj_b = np.asarray(inputs["proj_b"], np.float32)
    full = np.empty((BSZ, NW, WED), np.float32)
    for core in range(N_CORES):
        o = np.asarray(res.results[core]["out"], np.float32)  # [512, W] split order
        for s in range(SEQ_PER_CORE):
            b = core * SEQ_PER_CORE + s
            s_w, pl, off = meta[b]
            c = off + s_w                         # strip cols
            cols = np.where(c % 2 == 0, c // 2, J + c // 2)
            full[b] = o[:, cols].T
            if (pl == 0).any():
                full[b][pl == 0] = proj_b
    return full
